# revision 3
# baseline (speedup 1.0000x reference)
"""Trainium2 Bass kernel for nn_CMDPEncoder (VQ codebook quantize + random
batch-mix dequantize + DP noise).

Reference semantics:
    dots = einsum('bsd,vd->bsv', base, codebook)
    qi   = argmin_v(csq[v] - 2*dots)                  # [B,S]
    codes[b,s,j] = qi[rand_idx[b,s,j], s]
    out  = mean_j codebook[codes] + 0.1*noise

Sharding: split the sequence dim S across the 8 cores (64 positions each).
rand_idx mixing crosses only the batch dim at fixed s, so with S-sharding
every core's mixing is fully local.  Tokens are laid out s-major
(t = s_local*16 + b); the mix is a block-diagonal [128,128] matmul with
host-precomputed weights (counts/4 from rand_idx).

Scoring runs in fp16 (11-bit mantissa products are exact on the PE, fp32
accumulate).  On this problem's data the fp16 argmax matches the exact
argmax for all but ~1 token, and the exact top-2 rescore fixup (gather the
two candidate codebook rows, recompute exact fp32 scores on DVE, pick the
winner) covers a top-2 containment margin of ~0.27 vs fp16 noise ~0.02.

Matmuls are issued chunk-major over vblock groups so one LDWEIGHTS serves
3 matmuls; csq enters as a 2-row fp16 hi/lo matmul per vblock; the mix
matmul and its y-gather run in fp16 (counts/4 exact in fp16).
"""

import os
import sys

for p in ("/opt/trn_rl_repo",):
    if p not in sys.path:
        sys.path.insert(0, p)

import numpy as np

import concourse.bacc as bacc
import concourse.bass as bass
import concourse.mybir as mybir
import concourse.tile as tile
from concourse.bass_utils import run_bass_kernel_spmd

B, S, D, V, K = 16, 512, 768, 4096, 4
N_CORES = 8
SS = S // N_CORES            # 64 sequence positions per core
T = SS * B                   # 1024 tokens per core, t = s_local*16 + b
TT = T // 128                # 8 token tiles per core
KC = D // 128                # 6 contraction chunks
NV = V // 512                # 8 v-blocks
DP_EPSILON = 0.1
CSQ_CENTER = 768.0
DE = 776                     # padded cb_ext row: 768 cb + 1 csq + 7 pad
DEH = 776                    # fp16 mix-table row: 768 cb + 8 pad

F32 = mybir.dt.float32
F16 = mybir.dt.float16
U32 = mybir.dt.uint32
I32 = mybir.dt.int32

VGROUPS = [(0, 3), (3, 6), (6, 8)]   # vblock groups sharing LDWEIGHTS

_CACHED = {}


def _build_nc():
    nc = bacc.Bacc("TRN2", target_bir_lowering=False, debug=False,
                   num_devices=N_CORES)

    # xT: [128, (t, k, 128)] fp16, 2*x pre-tiled, contraction on partitions
    xT_d = nc.dram_tensor("xT", [128, KC * T], F16, kind="ExternalInput")
    # cbT: [128, (v, k, 512)] fp16
    cbT_d = nc.dram_tensor("cbT", [128, KC * V], F16, kind="ExternalInput")
    cbe_d = nc.dram_tensor("cbe", [V, DE], F32, kind="ExternalInput")
    cbeh_d = nc.dram_tensor("cbeh", [V, DEH], F16, kind="ExternalInput")
    csqL_d = nc.dram_tensor("csqL", [2, T], F16, kind="ExternalInput")
    csqR_d = nc.dram_tensor("csqR", [2, V], F16, kind="ExternalInput")
    w_d = nc.dram_tensor("w", [128, TT * 128], F16, kind="ExternalInput")
    noise_d = nc.dram_tensor("noise", [T, D], F32, kind="ExternalInput")
    xn_d = nc.dram_tensor("xn", [128, TT * D], F32, kind="ExternalInput")
    out_d = nc.dram_tensor("out", [T, D], F32, kind="ExternalOutput")

    XTW = KC * 128   # xt columns per token tile
    VBW = KC * 512   # cbT columns per v-block

    with tile.TileContext(nc) as tc:
        with (
            tc.tile_pool(name="big", bufs=1) as big,
            tc.tile_pool(name="work", bufs=2) as work,
            tc.tile_pool(name="sc", bufs=2) as sc_pool,
            tc.tile_pool(name="ypool", bufs=4) as ypool,
            tc.tile_pool(name="io", bufs=3) as io,
            tc.tile_pool(name="ps_s", bufs=6, space="PSUM") as ps_s,
            tc.tile_pool(name="ps_m", bufs=1, space="PSUM") as ps_m,
        ):
            # --- input staging, ordered so the first matmul can start early:
            # csq rows, xt tile 0, cb v-block 0 (in two halves), then the rest
            csql = big.tile([2, T], F16)
            csqr = big.tile([2, V], F16)
            nc.sync.dma_start(csql[:], csqL_d.ap())
            nc.sync.dma_start(csqr[:], csqR_d.ap())
            xt_t = []
            tl = big.tile([128, XTW], F16, tag="xt0")
            nc.sync.dma_start(tl[:], xT_d.ap()[:, 0:XTW])
            xt_t.append(tl)
            cb_t = []
            tl = big.tile([128, VBW], F16, tag="cbv0")
            nc.sync.dma_start(tl[:, 0:VBW // 2], cbT_d.ap()[:, 0:VBW // 2])
            nc.sync.dma_start(tl[:, VBW // 2:VBW],
                              cbT_d.ap()[:, VBW // 2:VBW])
            cb_t.append(tl)
            for v in range(1, NV):
                tl = big.tile([128, VBW], F16, tag=f"cbv{v}")
                nc.sync.dma_start(tl[:], cbT_d.ap()[:, v * VBW:(v + 1) * VBW])
                cb_t.append(tl)
            xn_t = []
            tl = big.tile([128, D], F32, tag="xn0")
            nc.sync.dma_start(tl[:], xn_d.ap()[:, 0:D])
            xn_t.append(tl)
            for t in range(1, TT):
                tl = big.tile([128, XTW], F16, tag=f"xt{t}")
                nc.sync.dma_start(tl[:], xT_d.ap()[:, t * XTW:(t + 1) * XTW])
                xt_t.append(tl)
                tl = big.tile([128, D], F32, tag=f"xn{t}")
                nc.sync.dma_start(tl[:], xn_d.ap()[:, t * D:(t + 1) * D])
                xn_t.append(tl)
            w = big.tile([128, TT * 128], F16)
            nc.sync.dma_start(w[:], w_d.ap())
            # last two tiles' noise pre-staged: their add runs on DVE instead
            # of the ACT-copy -> accum-DMA chain, shortening the tail
            nzlast = big.tile([128, 2 * D], F32)
            for a in range(2):
                tt_ = TT - 2 + a
                nc.sync.dma_start(nzlast[:, a * D:(a + 1) * D],
                                  noise_d.ap()[tt_ * 128:(tt_ + 1) * 128, :])

            def emit_scoring(t):
                """chunk-major: one LDWEIGHTS of xt[t][k] serves a whole
                vblock group; csq folds in as a 2-row fp16 matmul."""
                tsl = slice(t * 128, (t + 1) * 128)
                scores = sc_pool.tile([128, V], F32, tag="scores")
                for (v0, v1) in VGROUPS:
                    pss = [ps_s.tile([128, 512], F32, tag="ps_score",
                                     name=f"ps_{t}_{v0}_{i}")
                           for i in range(v1 - v0)]
                    for k in range(KC):
                        for i, v in enumerate(range(v0, v1)):
                            nc.tensor.matmul(
                                pss[i][:],
                                xt_t[t][:, k * 128:(k + 1) * 128],
                                cb_t[v][:, k * 512:(k + 1) * 512],
                                start=(k == 0), stop=False)
                    for i, v in enumerate(range(v0, v1)):
                        vsl = slice(v * 512, (v + 1) * 512)
                        nc.tensor.matmul(pss[i][:], csql[:, tsl],
                                         csqr[:, vsl], start=False, stop=True)
                        nc.scalar.copy(out=scores[:, vsl], in_=pss[i][:])
                return scores

            def emit_scan_fixup(t, scores):
                """argmax + exact top-2 rescore -> gather fp16 y row."""
                mx = work.tile([128, 8], F32, tag="mx")
                idx = work.tile([128, 8], U32, tag="idx")
                nc.vector.max(mx[:], scores[:])
                nc.vector.max_index(idx[:], mx[:], scores[:])

                xn = xn_t[t][:]
                cand = []
                for j in range(2):
                    cj = work.tile([128, 1], I32, tag=f"cand{j}")
                    nc.vector.tensor_copy(cj[:], idx[:, j:j + 1])
                    cand.append(cj)
                sj = []
                for j in range(2):
                    g = work.tile([128, DE], F32, tag=f"g{j}")
                    nc.gpsimd.indirect_dma_start(
                        out=g[:], out_offset=None, in_=cbe_d.ap(),
                        in_offset=bass.IndirectOffsetOnAxis(
                            ap=cand[j][:, :1], axis=0))
                    # NB: tensor_tensor_reduce hard-faults TRN2 here;
                    # scalar_tensor_tensor with accum_out does not.
                    tmp = work.tile([128, D], F32, tag="rescore_tmp")
                    dj = work.tile([128, 1], F32, tag=f"d{j}")
                    nc.vector.scalar_tensor_tensor(
                        out=tmp[:], in0=xn, scalar=1.0, in1=g[:, 0:D],
                        op0=mybir.AluOpType.bypass,
                        op1=mybir.AluOpType.mult, accum_out=dj[:])
                    s = work.tile([128, 1], F32, tag=f"s{j}")
                    # s = (dj * -2) + csq_cand
                    nc.vector.scalar_tensor_tensor(
                        out=s[:], in0=dj[:], scalar=-2.0, in1=g[:, D:D + 1],
                        op0=mybir.AluOpType.mult, op1=mybir.AluOpType.add)
                    sj.append(s)
                flip = work.tile([128, 1], I32, tag="flip")
                nc.vector.tensor_tensor(out=flip[:], in0=sj[1][:],
                                        in1=sj[0][:],
                                        op=mybir.AluOpType.is_lt)
                idx32 = work.tile([128, 1], I32, tag="idx32")
                nc.vector.tensor_copy(idx32[:], cand[0][:])
                nc.vector.copy_predicated(idx32[:], flip[:], cand[1][:])

                y = ypool.tile([128, DEH], F16, tag="y")
                nc.gpsimd.indirect_dma_start(
                    out=y[:], out_offset=None, in_=cbeh_d.ap(),
                    in_offset=bass.IndirectOffsetOnAxis(ap=idx32[:, :1], axis=0))
                return y

            def emit_output(t, y):
                """fp16 mix matmul -> ACT drain -> noise accum-DMA -> store."""
                tsl = slice(t * 128, (t + 1) * 128)
                pm = ps_m.tile([128, D], F32, tag="pm")
                nc.tensor.matmul(pm[:, 0:512], w[:, tsl], y[:, 0:512],
                                 start=True, stop=True)
                nc.tensor.matmul(pm[:, 512:D], w[:, tsl], y[:, 512:D],
                                 start=True, stop=True)
                ob = io.tile([128, D], F32, tag="out")
                if t >= TT - 2:
                    nz = nzlast[:, (t - (TT - 2)) * D:(t - (TT - 2) + 1) * D]
                    nc.vector.tensor_add(ob[:], pm[:], nz)
                else:
                    nc.scalar.copy(out=ob[:], in_=pm[:])
                    # add DP noise inline in the DMA (SWDGE accumulate)
                    nc.gpsimd.dma_start(out=ob[:], in_=noise_d.ap()[tsl, :],
                                        accum_op=mybir.AluOpType.add)
                nc.sync.dma_start(out_d.ap()[tsl, :], ob[:])

            # software pipeline: the scan/fixup/gather chain of tile t
            # overlaps scoring of t+1..; mixes trail by PIPE tiles.
            PIPE = 2
            pending = []
            for t in range(TT):
                scores = emit_scoring(t)
                y = emit_scan_fixup(t, scores)
                pending.append((t, y))
                if len(pending) > PIPE:
                    emit_output(*pending.pop(0))
            for item in pending:
                emit_output(*item)

    nc.compile()
    return nc


def _prep_inputs(base_embeddings, codebook, rand_idx, noise):
    """Build the 8 per-core input maps (all host-side numpy)."""
    x = np.ascontiguousarray(base_embeddings, dtype=np.float32)
    cb = np.ascontiguousarray(codebook, dtype=np.float32)
    ridx = np.asarray(rand_idx)
    nz = np.asarray(noise, dtype=np.float32)

    csq = (cb * cb).sum(-1, dtype=np.float32)              # [V]
    cbe = np.zeros((V, DE), np.float32)
    cbe[:, :D] = cb
    cbe[:, D] = csq
    cbeh = np.zeros((V, DEH), np.float16)
    cbeh[:, :D] = cb.astype(np.float16)
    csqc = (csq - CSQ_CENTER).astype(np.float32)
    r1 = csqc.astype(np.float16)
    r2 = (csqc - r1.astype(np.float32)).astype(np.float16)
    csqR = np.ascontiguousarray(np.stack([r1, r2]))        # [2, V] fp16
    csqL = np.full((2, T), -1.0, np.float16)

    # pre-tile [D, V] -> [128, (v, k, 512)] v-block-major layout, fp16
    cbT = cb.T.reshape(KC, 128, NV, 512).transpose(1, 2, 0, 3).reshape(128, KC * V)
    cbT = np.ascontiguousarray(cbT).astype(np.float16)

    shared = {"cbe": cbe, "cbeh": cbeh, "csqL": csqL, "csqR": csqR,
              "cbT": cbT}

    in_maps = []
    for c in range(N_CORES):
        ssl = slice(c * SS, (c + 1) * SS)
        # tokens t = s_local*16 + b
        xc = x[:, ssl, :].transpose(1, 0, 2).reshape(T, D)
        xT2 = (2.0 * xc).T                                 # [D, T] fp32
        xT2 = np.ascontiguousarray(
            xT2.reshape(KC, 128, TT, 128).transpose(1, 2, 0, 3)
            .reshape(128, KC * T)).astype(np.float16)
        nzc = np.ascontiguousarray(
            DP_EPSILON * nz[:, ssl, :].transpose(1, 0, 2).reshape(T, D))
        rc = ridx[:, ssl, :]                               # [B, SS, K]
        wm = np.zeros((TT, 128, 128), np.float32)
        for tt in range(TT):
            for g in range(8):
                s_local = tt * 8 + g
                r = rc[:, s_local, :]                      # [B, K] in [0,B)
                cnt = np.zeros((B, B), np.float32)         # [dst=b, src]
                for bdst in range(B):
                    np.add.at(cnt[bdst], r[bdst], 1.0)
                wm[tt, g * 16:(g + 1) * 16, g * 16:(g + 1) * 16] = cnt.T / K
        wm_t = np.ascontiguousarray(
            wm.transpose(1, 0, 2).reshape(128, TT * 128)).astype(np.float16)
        m = {"w": wm_t, "noise": nzc, "xT": xT2, **shared}
        m["xn"] = np.ascontiguousarray(
            xc.reshape(TT, 128, D).transpose(1, 0, 2).reshape(128, TT * D))
        in_maps.append(m)
    return in_maps


def kernel(base_embeddings, codebook, rand_idx, noise, _results_out=None):
    if "nc" not in _CACHED:
        _CACHED["nc"] = _build_nc()
    nc = _CACHED["nc"]
    in_maps = _prep_inputs(base_embeddings, codebook, rand_idx, noise)
    res = run_bass_kernel_spmd(nc, in_maps, list(range(N_CORES)))
    if _results_out is not None:
        _results_out.append(res)
    outs = []
    for c in range(N_CORES):
        oc = res.results[c]["out"].reshape(SS, B, D).transpose(1, 0, 2)
        outs.append(oc)
    return np.ascontiguousarray(np.concatenate(outs, axis=1))
